# revision 49
# baseline (speedup 1.0000x reference)
"""Trainium2 Bass kernel for nn_InvNUConv2d: label-grouped 1x1 conv.

  y[b, :, h, w] = weight[labels[b, h, w]] @ x[b, :, h, w] + bias[labels[b, h, w]]

Shapes (hardcoded): x [4, 16, 256, 256] f32, labels [4, 256, 256] i32,
weight [25, 16, 16] f32, bias [25] f32 (zeros).

Sharding: 8 cores, each takes half an image in H: core k -> (b = k//2,
h in [128*(k%2), 128*(k%2)+128)) = 32768 pixels x 16 channels.

Device algorithm per core (bf16 data path; rel-l2 ~3e-3 << 2e-2 gate):
  - x arrives as [128, 4096] bf16: partition (g*16+ch) holds channel ch of
    pixel group g (8 groups of 4096 pixels); pure layout reshape on host.
  - the slot space is split by raster QUARTER: each (label, quarter) gets a
    per-label-capacity sub-block, so each forward local_scatter has
    a <=2046-column dst (the 64KB GPSIMD scratch limit) AND a per-quarter
    data stream: every pixel is visited exactly once (4096 visits total).
  - per (label, half-pair): one bf16 matmul with block-diag(W_l^T x 8)
    stationary over quarters 2h,2h+1 via a strided rhs view (K=128
    packing, N=2*CAPS[l]): half-0 matmuls+copies overlap the Pool scatters
    of quarters 2,3 and inverse pass h waits only on its own half.
  - two inverse local_scatters restore raster order, pass h reading only
    quarters 2h,2h+1's contiguous slot slice (pads idx=-1 are dropped).
    The last NTAIL=4 raster columns ride appendix slots via plain DVE
    copies (identity maps need no Pool work) + per-column mixed-label
    matmuls, so 2x2046 inverse dst columns suffice.
  - direct DMA out; host undoes the layout reshape.

local_scatter (~2ns per visited column, vectorized through GPSIMD local
RAM) replaces ap_gather (~22ns/idx, per-element SBUF round trips) as the
permutation engine; the quarter decomposition + ragged per-label caps
minimize visits to 4096 (fwd) + ~6176 (inv), and the half-pipelined
schedule keeps the Pool engine gapless: ~214us baseline -> ~29us/iter.

Host does sharding/layout + index construction from labels; all data
movement and FLOPs run on the NeuronCores.
"""
import numpy as np
import ml_dtypes

import jax
import concourse.bacc as bacc
import concourse.bass as bass  # noqa: F401
import concourse.mybir as mybir
import concourse.tile as tile
from concourse import bass2jax
from jax.sharding import Mesh, PartitionSpec
from jax.experimental.shard_map import shard_map

B, C, H, W, L = 4, 16, 256, 256, 25
N_CORES = 8
NPIX = B * H * W // N_CORES  # 32768 pixels per core
NG = 8                       # partition groups (16 channels each)
GP = NPIX // NG              # 4096 pixels per group
MAX_ELEMS = 2046             # local_scatter dst columns per instruction
NTAIL = GP - 2 * MAX_ELEMS   # raster tail columns routed via appendix slots (4)
HALF = MAX_ELEMS             # raster half covered by one inverse-scatter pass
NS = 4                       # raster quarters (per-quarter slot regions)
QB = [0, 1022, 2046, 3070, 4092]  # quarter bounds (pairs sum to HALF=2046)
# per-label slot caps within a quarter region (exact seed-0 maxima; the
# runtime capacity check rebuilds the module if any input ever exceeds them)
CAPS = [62, 54, 58, 57, 61, 57, 56, 59, 57, 64, 57, 60, 58,
        63, 61, 61, 64, 58, 60, 60, 60, 62, 60, 63, 62]
OFF = list(np.cumsum([0] + CAPS[:-1]))  # label offsets within a region
CMAX = max(CAPS)             # psum tile sizing
REG = sum(CAPS)              # slot region per quarter (<= MAX_ELEMS)
M = NS * REG                 # slots per group
PH = M // 2                  # per-half slot stream (quarters 2h, 2h+1)
MA = M + NTAIL               # slot space incl appendix

F32 = mybir.dt.float32
BF16 = mybir.dt.bfloat16
I16 = mybir.dt.int16
BF16_NP = ml_dtypes.bfloat16


P3_CHUNKS = [(0, HALF), (HALF, HALF)]           # raster chunks (tail separate)
assert P3_CHUNKS[-1][0] + P3_CHUNKS[-1][1] + NTAIL == GP
assert REG <= MAX_ELEMS and all((QB[q + 1] - QB[q]) % 2 == 0 for q in range(NS))


def _set_caps(caps):
    """Raise slot capacities (module rebuild) if label counts ever exceed them."""
    global CAPS, OFF, CMAX, REG, M, PH, MA
    CAPS = list(caps)
    OFF = list(np.cumsum([0] + CAPS[:-1]))
    CMAX = max(CAPS)
    REG = sum(CAPS)
    M = NS * REG
    PH = M // 2
    MA = M + NTAIL
    assert REG % 2 == 0 and REG <= MAX_ELEMS, f"bad region size {REG}"


def _build_module(iters=1):
    nc = bacc.Bacc("TRN2", target_bir_lowering=False, debug=False, num_devices=N_CORES)
    x8 = nc.dram_tensor("x8", [128, GP], BF16, kind="ExternalInput").ap()
    p1i = [
        nc.dram_tensor(f"p1i{q}", [128, QB[q + 1] - QB[q]], I16, kind="ExternalInput").ap()
        for q in range(NS)
    ]
    p3i = [
        nc.dram_tensor(f"p3i{c}", [128, PH], I16, kind="ExternalInput").ap()
        for c in range(len(P3_CHUNKS))
    ]
    wbd = nc.dram_tensor("wbd", [L, 128, 128], BF16, kind="ExternalInput").ap()
    wfx = nc.dram_tensor("wfx", [NTAIL, 128, 128], BF16, kind="ExternalInput").ap()
    y8 = nc.dram_tensor("y8", [128, GP], BF16, kind="ExternalOutput").ap()

    with tile.TileContext(nc) as tc:
        with (
            tc.tile_pool(name="persist", bufs=1) as pp,
            tc.tile_pool(name="psum_mm", bufs=4, space="PSUM") as pmp,
        ):
            x_t = pp.tile([128, GP], BF16)
            p1_t = [
                pp.tile([128, QB[q + 1] - QB[q]], I16, name=f"p1t{q}")
                for q in range(NS)
            ]
            p3_t = [pp.tile([128, PH], I16, name=f"p3t{c}") for c in range(len(P3_CHUNKS))]
            wbd_t = pp.tile([128, L * 128], BF16)
            wfx_t = pp.tile([128, NTAIL * 128], BF16)
            xs = pp.tile([128, MA], BF16)
            ys = pp.tile([128, MA], BF16)
            yo = pp.tile([128, GP], BF16)
            nc.sync.dma_start(x_t[:], x8[:])
            for q in range(NS):
                nc.sync.dma_start(p1_t[q][:], p1i[q][:])
            for c in range(len(P3_CHUNKS)):
                nc.sync.dma_start(p3_t[c][:], p3i[c][:])
            for l in range(L):
                nc.sync.dma_start(wbd_t[:, l * 128 : (l + 1) * 128], wbd[l])
            for j in range(NTAIL):
                nc.sync.dma_start(wfx_t[:, j * 128 : (j + 1) * 128], wfx[j])

            # per-quarter region views; label sub-blocks are ragged slices
            xv = xs[:, :M].rearrange("p (q r) -> p q r", q=NS)
            yv = ys[:, :M].rearrange("p (q r) -> p q r", q=NS)
            for _ in range(iters):
                # phase 1: scatter each raster quarter into its own slot
                # region (one <=2046-column dst per quarter: every pixel is
                # visited exactly once). Matmuls+copies for quarters 2h,2h+1
                # are emitted between scatter pairs so PE/DVE work for half h
                # overlaps the Pool scatters of half h+1, and inverse pass h
                # only waits on its own half's copies.
                for q in range(2):
                    nc.gpsimd.local_scatter(
                        out_ap=xs[:, q * REG : (q + 1) * REG],
                        data_ap=x_t[:, QB[q] : QB[q + 1]],
                        idxs_ap=p1_t[q][:],
                        channels=128,
                        num_elems=REG,
                        num_idxs=QB[q + 1] - QB[q],
                    )
                # appendix: the tail columns are an identity move (DVE copy,
                # no Pool work), then per-column mixed-label matmuls
                nc.vector.tensor_copy(
                    out=xs[:, M:MA], in_=x_t[:, GP - NTAIL : GP]
                )
                pmf = pmp.tile([128, NTAIL], F32, tag="pmf")
                for j in range(NTAIL):
                    nc.tensor.matmul(
                        out=pmf[:, j : j + 1],
                        lhsT=wfx_t[:, j * 128 : (j + 1) * 128],
                        rhs=xs[:, M + j : M + j + 1],
                        start=True,
                        stop=True,
                    )
                nc.vector.tensor_copy(out=ys[:, M:MA], in_=pmf[:])
                # phase 2, half 0 (quarters 0,1) — overlaps P1 scatters 2,3
                for l in range(L):
                    pm = pmp.tile([128, 2 * CMAX], F32, tag="pm", name="pm")[:, : 2 * CAPS[l]]
                    nc.tensor.matmul(
                        out=pm,
                        lhsT=wbd_t[:, l * 128 : (l + 1) * 128],
                        rhs=xv[:, 0:2, OFF[l] : OFF[l] + CAPS[l]],
                        start=True,
                        stop=True,
                    )
                    nc.vector.tensor_copy(
                        out=yv[:, 0:2, OFF[l] : OFF[l] + CAPS[l]],
                        in_=pm.rearrange("p (q c) -> p q c", q=2),
                    )
                for q in range(2, NS):
                    nc.gpsimd.local_scatter(
                        out_ap=xs[:, q * REG : (q + 1) * REG],
                        data_ap=x_t[:, QB[q] : QB[q + 1]],
                        idxs_ap=p1_t[q][:],
                        channels=128,
                        num_elems=REG,
                        num_idxs=QB[q + 1] - QB[q],
                    )
                # phase 2, half 1 — overlaps inverse pass 0
                for l in range(L):
                    pm = pmp.tile([128, 2 * CMAX], F32, tag="pm", name="pm")[:, : 2 * CAPS[l]]
                    nc.tensor.matmul(
                        out=pm,
                        lhsT=wbd_t[:, l * 128 : (l + 1) * 128],
                        rhs=xv[:, 2:NS, OFF[l] : OFF[l] + CAPS[l]],
                        start=True,
                        stop=True,
                    )
                    nc.vector.tensor_copy(
                        out=yv[:, 2:NS, OFF[l] : OFF[l] + CAPS[l]],
                        in_=pm.rearrange("p (q c) -> p q c", q=2),
                    )
                # phase 3: scatter slots back to raster order (pads idx=-1);
                # pass h reads only quarters 2h,2h+1's contiguous slot slice.
                # appendix slots fill the last NTAIL raster columns (DVE copy)
                for c, (base, ln) in enumerate(P3_CHUNKS):
                    nc.gpsimd.local_scatter(
                        out_ap=yo[:, base : base + ln],
                        data_ap=ys[:, c * PH : (c + 1) * PH],
                        idxs_ap=p3_t[c][:],
                        channels=128,
                        num_elems=ln,
                        num_idxs=PH,
                    )
                nc.vector.tensor_copy(
                    out=yo[:, GP - NTAIL : GP], in_=ys[:, M:MA]
                )
            nc.sync.dma_start(y8[:], yo[:])
    nc.compile()
    return nc


def _make_runner(nc):
    bass2jax.install_neuronx_cc_hook()
    partition_name = nc.partition_id_tensor.name if nc.partition_id_tensor else None
    in_names, out_names, out_avals, zero_outs = [], [], [], []
    for alloc in nc.m.functions[0].allocations:
        if not isinstance(alloc, mybir.MemoryLocationSet):
            continue
        name = alloc.memorylocations[0].name
        if alloc.kind == "ExternalInput":
            if name != partition_name:
                in_names.append(name)
        elif alloc.kind == "ExternalOutput":
            shape = tuple(alloc.tensor_shape)
            dtype = mybir.dt.np(alloc.dtype)
            out_names.append(name)
            out_avals.append(jax.core.ShapedArray(shape, dtype))
            zero_outs.append(np.zeros(shape, dtype))
    n_params = len(in_names)
    in_names_full = in_names + out_names + ([partition_name] if partition_name else [])

    def _body(*args):
        operands = list(args)
        if partition_name is not None:
            operands.append(bass2jax.partition_id_tensor())
        outs = bass2jax._bass_exec_p.bind(
            *operands,
            out_avals=tuple(out_avals),
            in_names=tuple(in_names_full),
            out_names=tuple(out_names),
            lowering_input_output_aliases=(),
            sim_require_finite=False,
            sim_require_nnan=False,
            nc=nc,
        )
        return tuple(outs)

    devices = jax.devices()[:N_CORES]
    mesh = Mesh(np.asarray(devices), ("core",))
    nin = n_params + len(out_names)
    sharded = jax.jit(
        shard_map(
            _body,
            mesh=mesh,
            in_specs=(PartitionSpec("core"),) * nin,
            out_specs=(PartitionSpec("core"),) * len(out_names),
            check_rep=False,
        ),
        keep_unused=True,
    )

    def run(in_maps):
        per_core = [[np.asarray(m[name]) for name in in_names] for m in in_maps]
        concat_in = [
            np.concatenate([per_core[c][i] for c in range(N_CORES)], axis=0)
            for i in range(n_params)
        ]
        concat_zeros = [
            np.zeros((N_CORES * z.shape[0], *z.shape[1:]), z.dtype) for z in zero_outs
        ]
        out_arrs = sharded(*concat_in, *concat_zeros)
        out_arrs = [np.asarray(a) for a in out_arrs]
        return [
            {
                name: out_arrs[i].reshape(N_CORES, *out_avals[i].shape)[c]
                for i, name in enumerate(out_names)
            }
            for c in range(N_CORES)
        ]

    return run


_CACHE = {}


def _get_runner(iters=1):
    key = (iters, tuple(CAPS))
    if key not in _CACHE:
        nc = _build_module(iters)
        _CACHE[key] = _make_runner(nc)
    return _CACHE[key]


def _prep_core_inputs(xc, lc, weight):
    """xc [C, 128, W] f32, lc [128, W] i32 -> per-core input dict."""
    x_flat = np.asarray(xc).reshape(C, NPIX)  # pixel = h_local*W + w
    x8 = np.ascontiguousarray(
        x_flat.reshape(C, NG, GP).transpose(1, 0, 2).reshape(128, GP)
    ).astype(BF16_NP)
    lab = np.asarray(lc).reshape(NPIX)

    # per-quarter region-relative scatter indices (pixel -> OFF[l] + rank)
    p1idx = [
        np.full((NG, QB[q + 1] - QB[q]), -1, np.int16) for q in range(NS)
    ]
    # ys half-stream position -> chunk-relative raster (pads -1)
    ros = [np.full((NG, PH), -1, np.int16) for _ in range(2)]
    for g in range(NG):
        lg = lab[g * GP : (g + 1) * GP]
        off_arr = np.asarray(OFF, np.int64)
        caps_arr = np.asarray(CAPS, np.int64)
        for q in range(NS):
            qlen = QB[q + 1] - QB[q]
            seg = lg[QB[q] : QB[q + 1]]
            order = np.argsort(seg, kind="stable")
            counts = np.bincount(seg, minlength=L)
            if (counts > caps_arr).any():
                raise RuntimeError(f"label counts {counts} exceed CAPS")
            starts = np.zeros(L, np.int64)
            starts[1:] = np.cumsum(counts)[:-1]
            rank = np.empty(qlen, np.int64)
            rank[order] = np.arange(qlen) - np.repeat(starts, counts)
            pos = off_arr[seg] + rank  # region-relative slot
            p1idx[q][g] = pos.astype(np.int16)
            h, qh = q // 2, q % 2
            ros[h][g, qh * REG + pos] = (
                QB[q] - h * HALF + np.arange(qlen)
            ).astype(np.int16)
        # tail pixels live only in the appendix (idx -1 in all main chunks)

    out = {"x8": x8}
    for q in range(NS):
        out[f"p1i{q}"] = np.repeat(p1idx[q], 16, axis=0)
    for c in range(len(P3_CHUNKS)):
        out[f"p3i{c}"] = np.repeat(ros[c], 16, axis=0)

    wbd = np.zeros((L, 128, 128), np.float32)
    for l in range(L):
        wt = weight[l].T  # lhsT[(g,ch),(g,o)] = W[l, o, ch]
        for g in range(NG):
            wbd[l, g * 16 : g * 16 + 16, g * 16 : g * 16 + 16] = wt
    out["wbd"] = wbd.astype(BF16_NP)
    # per-tail-column stationaries: group g's block is the weight of the
    # label of that group's tail pixel
    wfx = np.zeros((NTAIL, 128, 128), np.float32)
    for j in range(NTAIL):
        for g in range(NG):
            l = int(lab[g * GP + GP - NTAIL + j])
            wfx[j, g * 16 : g * 16 + 16, g * 16 : g * 16 + 16] = weight[l].T
    out["wfx"] = wfx.astype(BF16_NP)
    return out


def kernel(x, labels, weight, bias):
    x = np.asarray(x, dtype=np.float32)
    labels = np.asarray(labels, dtype=np.int32)
    weight = np.asarray(weight, dtype=np.float32)
    bias = np.asarray(bias, dtype=np.float32)

    # capacity check: bump CAPS (and rebuild the module) if this input's
    # per-(group,label,quarter) counts exceed the seed-derived defaults
    mx = np.zeros(L, np.int64)
    for k in range(N_CORES):
        b, hh = k // 2, (k % 2) * 128
        lab = labels[b, hh : hh + 128, :].reshape(NG, GP)
        for g in range(NG):
            for q in range(NS):
                seg = lab[g, QB[q] : QB[q + 1]]
                mx = np.maximum(mx, np.bincount(seg, minlength=L))
    if (mx > np.asarray(CAPS)).any():
        caps = mx + 2
        if caps.sum() % 2:
            caps[int(np.argmin(caps))] += 1
        _set_caps(caps.tolist())

    run = _get_runner(1)  # cache keyed by (iters, CAPS)
    in_maps = []
    for k in range(N_CORES):
        b, hh = k // 2, (k % 2) * 128
        in_maps.append(
            _prep_core_inputs(x[b, :, hh : hh + 128, :], labels[b, hh : hh + 128, :], weight)
        )
    res = run(in_maps)

    y = np.empty((B, C, H, W), dtype=np.float32)
    for k in range(N_CORES):
        b, hh = k // 2, (k % 2) * 128
        yk = (
            res[k]["y8"]
            .astype(np.float32)
            .reshape(NG, C, GP)
            .transpose(1, 0, 2)
            .reshape(C, 128, W)
        )
        y[b, :, hh : hh + 128, :] = yk
    if np.any(bias):
        y += bias[labels][:, None, :, :]
    return y
